# revision 30
# baseline (speedup 1.0000x reference)
"""LIF spike kernel for Trainium2 (Bass/Tile), data-parallel over batch on 8 cores.

Reparametrized recurrence: v_t = u_t * 2^t with host-prescaled
x'_t = x_t * 2^t (exact power-of-2 scaling), so the step is
  v_{t+1} = (v_t <= 2^t) * v_t + x'_{t+1}
computed by ONE fused custom-DVE op (LIF_STEP_ANT). Spikes are emitted
as pair-packed ternary bytes by a second fused op (LIF_PACK2_ANT):
  byte_p = d(v_{2p}, 2^{2p}) + 3 * d(v_{2p+1}, 2^{2p+1}),
  d(v, th) = (v > th) - (v < th) in {-1, 0, 1}
so the output is 1 byte per 2 timesteps (2 MiB/core instead of 4) and
only the DVE computes. Host decodes d == 1 as spike.

Per-core layout: x [C=128, T=8, M=4096] f32 (M = B_loc*HW, prescaled),
out [C, 4, M] i8. The free dim runs as 2 interleaved chunks of 2048 so
each DVE op starts as soon as its 1 MiB input chunk lands (the t-major
in-stream then paces a dense DVE pipeline). t0/t1 fetches and the t0
steps are sub-split into 512-column slices, interleaved v0/x1, so the
first STEP starts after ~0.5 MiB of input. Out-DMAs go through the
(otherwise idle) Act engine's DGE so they never queue inside the input
FIFO.
"""

import numpy as np

import concourse.bacc as bacc
import concourse.mybir as mybir
from concourse.tile import TileContext
from concourse.bass_utils import run_bass_kernel_spmd

import concourse.dve_ops as dve_ops_mod
from concourse.dve_ops import DveOp, OPS, CUSTOM_DVE_SPECS
from concourse.dve_spec import Spec, Src0, Src1, C0, C1, C2, lower, _has_src1
from concourse.dve_uop import DveOpSpec

B, T, C, H, W = 32, 8, 128, 32, 32
HW = H * W
N_CORES = 8
B_LOC = B // N_CORES
M = B_LOC * HW  # free dim per (c, t): 4096
CH = 2048  # chunk width
NCH = M // CH

# sub-splits (within each chunk) for timesteps' fetches and ops beyond the
# hardcoded fine head (see subs())
SPLITS = {2: 2}

f32 = mybir.dt.float32
i8 = mybir.dt.int8
op = mybir.AluOpType

# odd timesteps t whose pair (t-1, t) bytes are produced by Act signs +
# GpSimd combine instead of the DVE pack op (dead: Pool engine cannot run
# ALU tensor ops on this toolchain — keep empty)
ACT_PAIRS_T = []


def _register(name, spec, subdim=False):
    existing = {op.name: op for op in OPS}
    if name in existing:
        return existing[name]
    row = dve_ops_mod._CUSTOM_DVE_ROW_BASE + len(OPS)
    assert row < 0x20, "no free custom-DVE rows"
    dve_ops_mod._SUB_OPCODE_FOR_NAME[name] = row
    shas = {}
    for ver in ("v3", "v4"):
        uops = lower(spec, ver=ver)
        shas[ver] = DveOpSpec(
            name=name, opcode=row, uops=uops, rd1_en=_has_src1(spec)
        ).sha(ver)
    op = DveOp(name, spec, subdim, uops_sha=shas)
    OPS.append(op)
    CUSTOM_DVE_SPECS[name] = spec
    return op


STEP = _register(
    "LIF_STEP_ANT",
    Spec(
        body=(Src0 * (Src0 <= C0)) + Src1,
        reference=lambda in0, in1, s0, s1, imm2: in0 * (in0 <= s0) + in1,
    ),
)

# single-source reset half of the step: m = v * (v <= th). Used for the
# stream-starved head timesteps so the DVE can work on v_t before x_{t+1}
# has arrived (the add then runs as a stock tensor_tensor).
RESET = _register(
    "LIF_RESET_ANT",
    Spec(
        body=Src0 * (Src0 <= C0),
        reference=lambda in0, in1, s0, s1, imm2: in0 * (in0 <= s0),
    ),
)

_d0 = (Src0 > C0) - (Src0 < C0)
_d1 = (Src1 > C1) - (Src1 < C1)
PACK2 = _register(
    "LIF_PACK2_ANT",
    Spec(
        body=_d0 + (_d1 * C2),
        reference=lambda in0, in1, s0, s1, imm2: (
            (in0 > s0).astype(np.float32)
            - (in0 < s0)
            + imm2 * ((in1 > s1).astype(np.float32) - (in1 < s1))
        ),
    ),
)

_nc_cache = None


def build_nc():
    nc = bacc.Bacc("TRN2", target_bir_lowering=False)
    x = nc.dram_tensor("x", [C, T, M], f32, kind="ExternalInput")
    out = nc.dram_tensor("out", [C, T // 2, M], i8, kind="ExternalOutput")

    with TileContext(nc) as tc:
        with (
            tc.tile_pool(name="xq", bufs=4) as xq,
            tc.tile_pool(name="vp", bufs=4) as vp,
            tc.tile_pool(name="op_", bufs=4) as opool,
            tc.tile_pool(name="sg", bufs=4) as sgpool,
            tc.tile_pool(name="cst", bufs=1) as cst,
        ):
            AF = mybir.ActivationFunctionType
            bias = {}
            for tp in ACT_PAIRS_T:
                for t in (tp - 1, tp):
                    bt = cst.tile([C, 1], f32, name=f"bias{t}")
                    nc.gpsimd.memset(bt[:], -float(2**t))
                    bias[t] = bt
            if ACT_PAIRS_T:
                warm = cst.tile([C, 1], i8, name="warm")
                nc.scalar.activation(
                    warm[:], bias[ACT_PAIRS_T[0]][:], AF.Sign,
                    bias=bias[ACT_PAIRS_T[0]][:],
                )
            def subs(t, ch):
                base = ch * CH
                if t <= 1 and ch == 0:
                    # fine head so the first STEP starts after ~0.5 MiB
                    return [(0, 512), (512, 2048)]
                n = SPLITS.get(t, 1)
                w = CH // n
                return [(base + k * w, base + (k + 1) * w) for k in range(n)]

            xt = {}

            def fetch(t):
                for ch in range(NCH):
                    xx = xq.tile([C, CH], f32, tag=f"x{ch}", name=f"x_{t}_{ch}")
                    for a0, a1 in subs(t, ch):
                        nc.sync.dma_start(
                            out=xx[:, a0 - ch * CH : a1 - ch * CH],
                            in_=x[:, t, a0:a1],
                        )
                    xt[(t, ch)] = xx

            # v_0 = x'_0 lands in the v pool; interleave v0/x1 sub-fetches so
            # the first STEP slice is ready after ~0.5 MiB of input
            v_cur = []
            x1t = []
            for ch in range(NCH):
                vt = vp.tile([C, CH], f32, tag=f"v{ch}", name=f"v0_{ch}")
                xx = xq.tile([C, CH], f32, tag=f"x{ch}", name=f"x_1_{ch}")
                v_cur.append(vt)
                x1t.append(xx)
                xt[(1, ch)] = xx
            for ch in range(NCH):
                for (a0, a1) in subs(0, ch):
                    o = slice(a0 - ch * CH, a1 - ch * CH)
                    nc.sync.dma_start(out=v_cur[ch][:, o], in_=x[:, 0, a0:a1])
                    nc.sync.dma_start(out=x1t[ch][:, o], in_=x[:, 1, a0:a1])
            fetch(2)

            v_prev = [None] * NCH
            deferred_outs = []
            for t in range(T):
                if t + 3 < T:
                    fetch(t + 3)
                v_now = list(v_cur)
                if t < T - 1:
                    for ch in range(NCH):
                        vn = vp.tile([C, CH], f32, tag=f"v{ch}", name=f"v{t + 1}_{ch}")
                        for a0, a1 in subs(t + 1, ch):
                            o = slice(a0 - ch * CH, a1 - ch * CH)
                            nc.vector._custom_dve(
                                STEP,
                                out=vn[:, o],
                                in0=v_now[ch][:, o],
                                in1=xt[(t + 1, ch)][:, o],
                                s0=float(2**t),
                            )
                        v_cur[ch] = vn
                if t % 2 == 1 and t in ACT_PAIRS_T:
                    # byte production off the DVE: Act ternary signs,
                    # GpSimd combine sign_b*3 + sign_a
                    for ch in range(NCH):
                        sa = sgpool.tile([C, CH], i8, tag=f"s{ch}", name=f"sa{t}_{ch}")
                        sb = sgpool.tile([C, CH], i8, tag=f"s{ch}", name=f"sb{t}_{ch}")
                        nc.scalar.activation(
                            sa[:], v_prev[ch][:], AF.Sign, bias=bias[t - 1][:]
                        )
                        nc.scalar.activation(
                            sb[:], v_now[ch][:], AF.Sign, bias=bias[t][:]
                        )
                        ob = opool.tile([C, CH], i8, tag=f"o{ch}", name=f"o{t // 2}_{ch}")
                        nc.gpsimd.scalar_tensor_tensor(
                            ob[:], sb[:], 3.0, sa[:], op.mult, op.add
                        )
                        nc.scalar.dma_start(
                            out=out[:, t // 2, ch * CH : (ch + 1) * CH],
                            in_=ob[:],
                        )
                elif t % 2 == 1:
                    # sub-split the final pack so its out-DMA starts earlier
                    nsub = 2 if t == T - 1 else 1
                    w = CH // nsub
                    for ch in range(NCH):
                        ob = opool.tile([C, CH], i8, tag=f"o{ch}", name=f"o{t // 2}_{ch}")
                        for k in range(nsub):
                            o = slice(k * w, (k + 1) * w)
                            nc.vector._custom_dve(
                                PACK2,
                                out=ob[:, o],
                                in0=v_prev[ch][:, o],
                                in1=v_now[ch][:, o],
                                s0=float(2 ** (t - 1)),
                                s1=float(2**t),
                                imm2=3.0,
                            )
                            # defer the out-DMA so it never steals DMA-engine
                            # bandwidth from the input stream mid-flight; all
                            # outs drain in the post-input tail window
                            deferred_outs.append(
                                (
                                    out[:, t // 2, ch * CH + k * w : ch * CH + (k + 1) * w],
                                    ob[:, o],
                                )
                            )
                v_prev = v_now
            for dst, src in deferred_outs:
                nc.scalar.dma_start(out=dst, in_=src)
    nc.compile()
    return nc


def make_in_maps(x: np.ndarray) -> list[dict]:
    xs = np.ascontiguousarray(x).reshape(B, T, C, HW)
    scale = (2.0 ** np.arange(T, dtype=np.float32)).astype(np.float32)
    xs = (xs * scale[None, :, None, None]).astype(np.float32)
    return [
        {
            "x": np.ascontiguousarray(
                xs[i * B_LOC : (i + 1) * B_LOC].transpose(2, 1, 0, 3)
            ).reshape(C, T, M)
        }
        for i in range(N_CORES)
    ]


def kernel(x: np.ndarray) -> np.ndarray:
    global _nc_cache
    if _nc_cache is None:
        _nc_cache = build_nc()
    res = run_bass_kernel_spmd(_nc_cache, make_in_maps(x), list(range(N_CORES)))
    parts = []
    for i in range(N_CORES):
        raw = res.results[i]["out"].reshape(C, T // 2, B_LOC, HW).astype(np.int16)
        r = raw + 4  # (d_e + 1) + 3 * (d_o + 1) in [0, 8]
        s = np.empty((T, C, B_LOC, HW), dtype=bool)
        for p in range(T // 2):
            s[2 * p] = r[:, p] % 3 == 2
            s[2 * p + 1] = r[:, p] // 3 == 2
        parts.append(s.transpose(2, 0, 1, 3))  # [B_LOC, T, C, HW]
    full = np.concatenate(parts, axis=0)
    return full.reshape(B, T, C, H, W).astype(np.float32)


# revision 31
# speedup vs baseline: 1.0262x; 1.0262x over previous
"""LIF spike kernel for Trainium2 (Bass/Tile), data-parallel over batch on 8 cores.

Reparametrized recurrence: v_t = u_t * 2^t with host-prescaled
x'_t = x_t * 2^t (exact power-of-2 scaling), so the step is
  v_{t+1} = (v_t <= 2^t) * v_t + x'_{t+1}
computed by ONE fused custom-DVE op (LIF_STEP_ANT). Spikes are emitted
as pair-packed ternary bytes by a second fused op (LIF_PACK2_ANT):
  byte_p = d(v_{2p}, 2^{2p}) + 3 * d(v_{2p+1}, 2^{2p+1}),
  d(v, th) = (v > th) - (v < th) in {-1, 0, 1}
so the output is 1 byte per 2 timesteps (2 MiB/core instead of 4) and
only the DVE computes. Host decodes d == 1 as spike.

Per-core layout: x [C=128, T=8, M=4096] f32 (M = B_loc*HW, prescaled),
out [C, 4, M] i8. The free dim runs as 2 interleaved chunks of 2048 so
each DVE op starts as soon as its 1 MiB input chunk lands (the t-major
in-stream then paces a dense DVE pipeline). t0/t1 fetches and the t0
steps are sub-split into 512-column slices, interleaved v0/x1, so the
first STEP starts after ~0.5 MiB of input. Out-DMAs go through the
(otherwise idle) Act engine's DGE so they never queue inside the input
FIFO.
"""

import numpy as np

import concourse.bacc as bacc
import concourse.mybir as mybir
from concourse.tile import TileContext
from concourse.bass_utils import run_bass_kernel_spmd

import concourse.dve_ops as dve_ops_mod
from concourse.dve_ops import DveOp, OPS, CUSTOM_DVE_SPECS
from concourse.dve_spec import Spec, Src0, Src1, C0, C1, C2, lower, _has_src1
from concourse.dve_uop import DveOpSpec

B, T, C, H, W = 32, 8, 128, 32, 32
HW = H * W
N_CORES = 8
B_LOC = B // N_CORES
M = B_LOC * HW  # free dim per (c, t): 4096
CH = 2048  # chunk width
NCH = M // CH

# sub-splits (within each chunk) for timesteps' fetches and ops beyond the
# hardcoded fine head (see subs())
SPLITS = {}

f32 = mybir.dt.float32
i8 = mybir.dt.int8
op = mybir.AluOpType

# odd timesteps t whose pair (t-1, t) bytes are produced by Act signs +
# GpSimd combine instead of the DVE pack op (dead: Pool engine cannot run
# ALU tensor ops on this toolchain — keep empty)
ACT_PAIRS_T = []


def _register(name, spec, subdim=False):
    existing = {op.name: op for op in OPS}
    if name in existing:
        return existing[name]
    row = dve_ops_mod._CUSTOM_DVE_ROW_BASE + len(OPS)
    assert row < 0x20, "no free custom-DVE rows"
    dve_ops_mod._SUB_OPCODE_FOR_NAME[name] = row
    shas = {}
    for ver in ("v3", "v4"):
        uops = lower(spec, ver=ver)
        shas[ver] = DveOpSpec(
            name=name, opcode=row, uops=uops, rd1_en=_has_src1(spec)
        ).sha(ver)
    op = DveOp(name, spec, subdim, uops_sha=shas)
    OPS.append(op)
    CUSTOM_DVE_SPECS[name] = spec
    return op


STEP = _register(
    "LIF_STEP_ANT",
    Spec(
        body=(Src0 * (Src0 <= C0)) + Src1,
        reference=lambda in0, in1, s0, s1, imm2: in0 * (in0 <= s0) + in1,
    ),
)

# single-source reset half of the step: m = v * (v <= th). Used for the
# stream-starved head timesteps so the DVE can work on v_t before x_{t+1}
# has arrived (the add then runs as a stock tensor_tensor).
RESET = _register(
    "LIF_RESET_ANT",
    Spec(
        body=Src0 * (Src0 <= C0),
        reference=lambda in0, in1, s0, s1, imm2: in0 * (in0 <= s0),
    ),
)

_d0 = (Src0 > C0) - (Src0 < C0)
_d1 = (Src1 > C1) - (Src1 < C1)
PACK2 = _register(
    "LIF_PACK2_ANT",
    Spec(
        body=_d0 + (_d1 * C2),
        reference=lambda in0, in1, s0, s1, imm2: (
            (in0 > s0).astype(np.float32)
            - (in0 < s0)
            + imm2 * ((in1 > s1).astype(np.float32) - (in1 < s1))
        ),
    ),
)

_nc_cache = None


def build_nc():
    nc = bacc.Bacc("TRN2", target_bir_lowering=False)
    x = nc.dram_tensor("x", [C, T, M], f32, kind="ExternalInput")
    out = nc.dram_tensor("out", [C, T // 2, M], i8, kind="ExternalOutput")

    with TileContext(nc) as tc:
        with (
            tc.tile_pool(name="xq", bufs=4) as xq,
            tc.tile_pool(name="vp", bufs=4) as vp,
            tc.tile_pool(name="op_", bufs=4) as opool,
            tc.tile_pool(name="sg", bufs=4) as sgpool,
            tc.tile_pool(name="cst", bufs=1) as cst,
        ):
            AF = mybir.ActivationFunctionType
            bias = {}
            for tp in ACT_PAIRS_T:
                for t in (tp - 1, tp):
                    bt = cst.tile([C, 1], f32, name=f"bias{t}")
                    nc.gpsimd.memset(bt[:], -float(2**t))
                    bias[t] = bt
            if ACT_PAIRS_T:
                warm = cst.tile([C, 1], i8, name="warm")
                nc.scalar.activation(
                    warm[:], bias[ACT_PAIRS_T[0]][:], AF.Sign,
                    bias=bias[ACT_PAIRS_T[0]][:],
                )
            def subs(t, ch):
                base = ch * CH
                if t <= 1 and ch == 0:
                    # fine head so the first STEP starts after ~0.5 MiB
                    return [(0, 512), (512, 2048)]
                n = SPLITS.get(t, 1)
                w = CH // n
                return [(base + k * w, base + (k + 1) * w) for k in range(n)]

            xt = {}

            def fetch(t):
                for ch in range(NCH):
                    xx = xq.tile([C, CH], f32, tag=f"x{ch}", name=f"x_{t}_{ch}")
                    for a0, a1 in subs(t, ch):
                        nc.sync.dma_start(
                            out=xx[:, a0 - ch * CH : a1 - ch * CH],
                            in_=x[:, t, a0:a1],
                        )
                    xt[(t, ch)] = xx

            # v_0 = x'_0 lands in the v pool; interleave v0/x1 sub-fetches so
            # the first STEP slice is ready after ~0.5 MiB of input
            v_cur = []
            x1t = []
            for ch in range(NCH):
                vt = vp.tile([C, CH], f32, tag=f"v{ch}", name=f"v0_{ch}")
                xx = xq.tile([C, CH], f32, tag=f"x{ch}", name=f"x_1_{ch}")
                v_cur.append(vt)
                x1t.append(xx)
                xt[(1, ch)] = xx
            for ch in range(NCH):
                for (a0, a1) in subs(0, ch):
                    o = slice(a0 - ch * CH, a1 - ch * CH)
                    nc.sync.dma_start(out=v_cur[ch][:, o], in_=x[:, 0, a0:a1])
                    nc.sync.dma_start(out=x1t[ch][:, o], in_=x[:, 1, a0:a1])
            fetch(2)

            v_prev = [None] * NCH
            deferred_outs = []
            for t in range(T):
                if t + 3 < T:
                    fetch(t + 3)
                v_now = list(v_cur)
                if t < T - 1:
                    for ch in range(NCH):
                        vn = vp.tile([C, CH], f32, tag=f"v{ch}", name=f"v{t + 1}_{ch}")
                        for a0, a1 in subs(t + 1, ch):
                            o = slice(a0 - ch * CH, a1 - ch * CH)
                            nc.vector._custom_dve(
                                STEP,
                                out=vn[:, o],
                                in0=v_now[ch][:, o],
                                in1=xt[(t + 1, ch)][:, o],
                                s0=float(2**t),
                            )
                        v_cur[ch] = vn
                if t % 2 == 1 and t in ACT_PAIRS_T:
                    # byte production off the DVE: Act ternary signs,
                    # GpSimd combine sign_b*3 + sign_a
                    for ch in range(NCH):
                        sa = sgpool.tile([C, CH], i8, tag=f"s{ch}", name=f"sa{t}_{ch}")
                        sb = sgpool.tile([C, CH], i8, tag=f"s{ch}", name=f"sb{t}_{ch}")
                        nc.scalar.activation(
                            sa[:], v_prev[ch][:], AF.Sign, bias=bias[t - 1][:]
                        )
                        nc.scalar.activation(
                            sb[:], v_now[ch][:], AF.Sign, bias=bias[t][:]
                        )
                        ob = opool.tile([C, CH], i8, tag=f"o{ch}", name=f"o{t // 2}_{ch}")
                        nc.gpsimd.scalar_tensor_tensor(
                            ob[:], sb[:], 3.0, sa[:], op.mult, op.add
                        )
                        nc.scalar.dma_start(
                            out=out[:, t // 2, ch * CH : (ch + 1) * CH],
                            in_=ob[:],
                        )
                elif t % 2 == 1:
                    # sub-split the final pack so its out-DMA starts earlier
                    nsub = 2 if t == T - 1 else 1
                    w = CH // nsub
                    for ch in range(NCH):
                        ob = opool.tile([C, CH], i8, tag=f"o{ch}", name=f"o{t // 2}_{ch}")
                        for k in range(nsub):
                            o = slice(k * w, (k + 1) * w)
                            nc.vector._custom_dve(
                                PACK2,
                                out=ob[:, o],
                                in0=v_prev[ch][:, o],
                                in1=v_now[ch][:, o],
                                s0=float(2 ** (t - 1)),
                                s1=float(2**t),
                                imm2=3.0,
                            )
                            # defer the out-DMA so it never steals DMA-engine
                            # bandwidth from the input stream mid-flight; all
                            # outs drain in the post-input tail window
                            deferred_outs.append(
                                (
                                    out[:, t // 2, ch * CH + k * w : ch * CH + (k + 1) * w],
                                    ob[:, o],
                                )
                            )
                v_prev = v_now
            for dst, src in deferred_outs:
                nc.scalar.dma_start(out=dst, in_=src)
    nc.compile()
    return nc


def make_in_maps(x: np.ndarray) -> list[dict]:
    xs = np.ascontiguousarray(x).reshape(B, T, C, HW)
    scale = (2.0 ** np.arange(T, dtype=np.float32)).astype(np.float32)
    xs = (xs * scale[None, :, None, None]).astype(np.float32)
    return [
        {
            "x": np.ascontiguousarray(
                xs[i * B_LOC : (i + 1) * B_LOC].transpose(2, 1, 0, 3)
            ).reshape(C, T, M)
        }
        for i in range(N_CORES)
    ]


def kernel(x: np.ndarray) -> np.ndarray:
    global _nc_cache
    if _nc_cache is None:
        _nc_cache = build_nc()
    res = run_bass_kernel_spmd(_nc_cache, make_in_maps(x), list(range(N_CORES)))
    parts = []
    for i in range(N_CORES):
        raw = res.results[i]["out"].reshape(C, T // 2, B_LOC, HW).astype(np.int16)
        r = raw + 4  # (d_e + 1) + 3 * (d_o + 1) in [0, 8]
        s = np.empty((T, C, B_LOC, HW), dtype=bool)
        for p in range(T // 2):
            s[2 * p] = r[:, p] % 3 == 2
            s[2 * p + 1] = r[:, p] // 3 == 2
        parts.append(s.transpose(2, 0, 1, 3))  # [B_LOC, T, C, HW]
    full = np.concatenate(parts, axis=0)
    return full.reshape(B, T, C, H, W).astype(np.float32)
